# revision 11
# baseline (speedup 1.0000x reference)
"""Trainium2 Bass kernel for multi-head self-attention (v2).

Problem: B=4, T=2048, D=1024, H=16 heads (dh=64), causal, fp32.

Sharding (8 cores): core c -> (batch c % 4, head-group c // 4). Each core
computes one batch's 8 heads (tensor parallel over heads): QKV projection
for its head-group, attention, and a partial output projection (W_out
row-shard). The host sums the two head-group partials per batch and adds
b_out.

v2 changes vs v1:
 - all matmul operands bf16 (x/W_qkv/W_out converted on host; QKV output
   and attention y quantized to bf16 on the device) -> halves input DMA
   and SBUF, full-rate MMs with FWL weight loads.
 - attention inner loop restructured as chunk pipeline per (head-pair,
   query-chunk): scores MMs -> ONE wide exp (N=2048 or 1024) -> AV MMs
   accumulate immediately; eT only lives per-chunk (4KB vs 32KB).
 - causal diag handled by multiplicative bf16 triangle mask on the exp
   OUTPUT (DVE 2x rate) instead of additive -1e30 PSUM mask.
 - out-projection tiles emitted early (tt 0..3 after qc0, etc.) to fill
   PE gaps while ACT runs exp; out DMA goes directly from PSUM.
"""

import os
import sys

sys.path.insert(0, "/opt/trn_rl_repo")

import numpy as np
import ml_dtypes

import concourse.bass as bass
import concourse.tile as tile
from concourse import bacc, mybir
from concourse.bass_utils import run_bass_kernel_spmd

B, T, D, H = 4, 2048, 1024, 16
DH = D // H          # 64
HL = H // 2          # 8 local heads per core
DL = HL * DH         # 512 local head dims
NT = T // 128        # 16 t-tiles of 128
NKI = D // 128       # 8 contraction tiles for QKV
NPAIR = HL // 2      # 4 head pairs

F32 = mybir.dt.float32
BF16 = mybir.dt.bfloat16

CHUNK2 = False       # 2 kts per scores/exp chunk (N=2048) vs 1 (N=1024)

_CACHED = {}


def _emit(nc, tc, causal):
    xT = nc.dram_tensors["xT"].ap()
    w_qkv = nc.dram_tensors["w_qkv"].ap()
    b_qkv = nc.dram_tensors["b_qkv"].ap()
    w_out = nc.dram_tensors["w_out"].ap()
    out = nc.dram_tensors["out"].ap()

    xT_r = xT.rearrange("(ko ki) t -> ki ko t", ki=128)
    w_r = w_qkv.rearrange("(ko ki) n -> ki ko n", ki=128)
    scale = float(1.0 / np.sqrt(DH))

    with (
        tc.tile_pool(name="const", bufs=1) as cpool,
        tc.tile_pool(name="qkvT", bufs=1) as qpool,
        tc.tile_pool(name="yT", bufs=1) as ypool,
        tc.tile_pool(name="expT", bufs=8) as epool,
        tc.tile_pool(name="rec", bufs=2) as rpool,
        tc.tile_pool(name="ycopy", bufs=3) as ycpool,
        tc.tile_pool(name="yp", bufs=2, space="PSUM") as yp,
        tc.tile_pool(name="sp", bufs=(1 if CHUNK2 else 2), space="PSUM") as sp,
        tc.tile_pool(name="ps512", bufs=2, space="PSUM") as ps512,
    ):
        QT = qpool.tile([128, NPAIR, T], BF16, tag="QT")  # [d-pair, pair, t]
        KT = qpool.tile([128, NPAIR, T], BF16, tag="KT")
        V = qpool.tile([128, NT, HL, DH + 1], BF16, tag="V")
        yT = ypool.tile([128, NPAIR, T], BF16, tag="yT")
        nc.vector.memset(V[:, :, :, DH], 1.0)

        # constants: multiplicative bf16 triangle mask (1 where k<=q local,
        # 0 where k>q), duplicated for the 2 heads of a pair; ones; biases
        ones1 = cpool.tile([128, 1], F32, tag="ones1")
        nc.vector.memset(ones1[:], 1.0)
        tri = cpool.tile([128, 2, 128], BF16, tag="tri")
        nc.vector.memset(tri[:], 1.0)
        for i in range(2):
            nc.gpsimd.affine_select(
                out=tri[:, i],
                in_=tri[:, i],
                compare_op=mybir.AluOpType.is_ge,
                fill=0.0,
                base=0,
                pattern=[[1, 128]],
                channel_multiplier=-1,
            )
        bqk = []
        for c in range(8):
            bt = cpool.tile([128, 1], F32, tag=f"bqk{c}")
            nc.sync.dma_start(
                bt[:], b_qkv[c * 128 : (c + 1) * 128].rearrange("(p o) -> p o", o=1)
            )
            bqk.append(bt)
        bv1 = cpool.tile([1, DL], F32, tag="bv1")
        nc.sync.dma_start(
            bv1[:], b_qkv[2 * DL : 3 * DL].rearrange("(o n) -> o n", o=1)
        )
        bv = cpool.tile([128, DL], F32, tag="bv")
        nc.gpsimd.partition_broadcast(bv[:], bv1[:])

        def attn_pair(p, qc):
            """Chunked scores->exp->AV pipeline for head pair (2p, 2p+1)."""
            QT0, QT1 = QT[0:64, p], QT[64:128, p]
            KT0, KT1 = KT[0:64, p], KT[64:128, p]
            h0, h1 = 2 * p, 2 * p + 1
            nkt = 4 * qc + 4 if causal else NT
            ndiag = 4 if causal else 0
            qlo = qc * 512
            yps0 = yp.tile([65, 512], F32, tag="yp", name=f"yp_{h0}_{qc}")
            yps1 = yp.tile([65, 512], F32, tag="yp", name=f"yp_{h1}_{qc}")

            def scores_mm(ps_slice, kts, qoffs=None):
                for i, kt in enumerate(kts):
                    qa = qlo + (qoffs[i] if qoffs else 0)
                    nc.tensor.matmul(
                        ps_slice(i, 0),
                        KT0[:, kt * 128 : (kt + 1) * 128],
                        QT0[:, qa : qlo + 512],
                        start=True,
                        stop=True,
                    )
                    nc.tensor.matmul(
                        ps_slice(i, 1),
                        KT1[:, kt * 128 : (kt + 1) * 128],
                        QT1[:, qa : qlo + 512],
                        start=True,
                        stop=True,
                    )

            def av_mm(eTc, i, kt, rq=0):
                last = kt == nkt - 1
                nc.tensor.matmul(
                    yps0[:, rq:],
                    V[:, kt, h0],
                    eTc[:, i, 0, rq:],
                    start=(kt == 0),
                    stop=last,
                )
                nc.tensor.matmul(
                    yps1[:, rq:],
                    V[:, kt, h1],
                    eTc[:, i, 1, rq:],
                    start=(kt == 0),
                    stop=last,
                )

            def scores_exp(kt, r):
                """Scores pair MMs -> exp (-> tri mask) for one kt."""
                rq = 0 if r is None else r * 128
                valid = 512 - rq
                ps = sp.tile(
                    [128, 1, 2, 512], F32, tag="sp", name=f"sp_{h0}_{qc}_{kt}"
                )
                scores_mm(
                    lambda i, h: ps[:, 0, h, :valid],
                    [kt],
                    qoffs=[rq] if r is not None else None,
                )
                eTc = epool.tile(
                    [128, 1, 2, 512], BF16, tag="eTc", name=f"eTc_{h0}_{qc}_{kt}"
                )
                nc.scalar.activation(
                    eTc[:, 0, :, rq:],
                    ps[:, 0, :, :valid],
                    mybir.ActivationFunctionType.Exp,
                    scale=scale,
                )
                if r is not None:
                    nc.vector.tensor_tensor(
                        eTc[:, 0, :, rq : rq + 128],
                        eTc[:, 0, :, rq : rq + 128],
                        tri[:],
                        mybir.AluOpType.mult,
                    )
                return eTc

            # software pipeline: AV trails scores/exp by two kts so the PE
            # queue never waits on ACT exp (or the diag mask on DVE)
            kts = [(kt, None) for kt in range(nkt - ndiag)]
            kts += [(nkt - ndiag + r, r) for r in range(ndiag)]
            pq = []
            for kt, r in kts:
                eTc = scores_exp(kt, r)
                if len(pq) == 2:
                    e0, k0, r0 = pq.pop(0)
                    av_mm(e0, 0, k0, rq=0 if r0 is None else r0 * 128)
                pq.append((eTc, kt, r))
            for e0, k0, r0 in pq:
                av_mm(e0, 0, k0, rq=0 if r0 is None else r0 * 128)

            # stage yps -> SBUF on ACT so the yp banks recycle immediately;
            # reciprocal/broadcast/normalize then run fully off-path
            yc = ycpool.tile([65, 2, 512], F32, tag="yc", name=f"yc_{p}_{qc}")
            nc.vector.tensor_copy(yc[:, 0], yps0[:])
            nc.vector.tensor_copy(yc[:, 1], yps1[:])
            for par in (0, 1):
                rec = rpool.tile([1, 512], F32, tag="rec")
                nc.vector.reciprocal(rec[:], yc[64:65, par, :])
                rbc = rpool.tile([64, 512], F32, tag="rbc")
                nc.gpsimd.partition_broadcast(rbc[:], rec[:])
                nc.vector.tensor_tensor(
                    yT[par * 64 : par * 64 + 64, p, qlo : qlo + 512],
                    yc[:64, par, :],
                    rbc[:],
                    mybir.AluOpType.mult,
                )

        # ---- Phase A: QKV projection rounds (+ attention interleaved) ----
        with (
            tc.tile_pool(name="xw", bufs=1) as wpool,
            tc.tile_pool(name="xstream", bufs=2) as xwpool,
            tc.tile_pool(name="ostg", bufs=3) as opool,
        ):
            wchs = []
            for c in range(8):
                wch = wpool.tile([128, NKI, 128], BF16, tag=f"wch{c}")
                nc.scalar.dma_start(wch[:], w_r[:, :, c * 128 : (c + 1) * 128])
                wchs.append(wch)
            wv_sb = wpool.tile([128, NKI, DL], BF16, tag="wv")
            nc.scalar.dma_start(wv_sb[:], w_r[:, :, 2 * DL : 3 * DL])
            wo_sb = wpool.tile([128, NPAIR, D], BF16, tag="wo")
            for j in range(NPAIR):
                nc.scalar.dma_start(wo_sb[:, j], w_out[j * 128 : (j + 1) * 128, :])

            def tc_round(tcx):
                xc = xwpool.tile([128, NKI, 512], BF16, tag="xc")
                nc.sync.dma_start(
                    xc[:], xT_r[:, :, tcx * 512 : (tcx + 1) * 512]
                )
                for c in range(8):  # Q/K channel tiles
                    dstT = QT if c < 4 else KT
                    ps = ps512.tile([128, 512], F32, tag="ps512")
                    for kt in range(NKI):
                        nc.tensor.matmul(
                            ps[:],
                            wchs[c][:, kt],
                            xc[:, kt],
                            start=(kt == 0),
                            stop=(kt == NKI - 1),
                        )
                    dst = dstT[:, c % 4, tcx * 512 : (tcx + 1) * 512]
                    nc.vector.tensor_scalar_add(dst, ps[:], bqk[c][:])
                for tt in range(4 * tcx, 4 * tcx + 4):  # V t-tiles
                    ps2 = ps512.tile([128, DL], F32, tag="ps512")
                    for kt in range(NKI):
                        nc.tensor.matmul(
                            ps2[:],
                            xc[:, kt, (tt % 4) * 128 : (tt % 4 + 1) * 128],
                            wv_sb[:, kt],
                            start=(kt == 0),
                            stop=(kt == NKI - 1),
                        )
                    nc.vector.tensor_tensor(
                        V[:, tt, :, :DH],
                        ps2.rearrange("p (h d) -> p h d", h=HL),
                        bv.rearrange("p (h d) -> p h d", h=HL),
                        mybir.AluOpType.add,
                    )

            def out_tt(tt):
                """Output-projection rows for one t-tile."""
                for n in range(2):
                    ps = ps512.tile([128, 512], F32, tag="ps512", name=f"psC_{tt}_{n}")
                    for j in range(NPAIR):
                        nc.tensor.matmul(
                            ps[:],
                            yT[:, j, tt * 128 : (tt + 1) * 128],
                            wo_sb[:, j, n * 512 : (n + 1) * 512],
                            start=(j == 0),
                            stop=(j == NPAIR - 1),
                        )
                    stg = opool.tile([128, 512], F32, tag="ostg", name=f"stg_{tt}_{n}")
                    nc.vector.tensor_copy(stg[:], ps[:])
                    nc.sync.dma_start(
                        out[tt * 128 : (tt + 1) * 128, n * 512 : (n + 1) * 512],
                        stg[:],
                    )

            tc_round(0)
            if causal:
                for p in range(NPAIR):
                    attn_pair(p, 0)
            tc_round(1)
            if causal:
                for p in range(NPAIR):
                    attn_pair(p, 1)
                for tt in range(0, 4):
                    out_tt(tt)
            tc_round(2)
            if causal:
                for p in range(NPAIR):
                    attn_pair(p, 2)
                for tt in range(4, 8):
                    out_tt(tt)
            tc_round(3)
            if not causal:
                for qc in range(3):
                    for p in range(NPAIR):
                        attn_pair(p, qc)
                for tt in range(0, 8):
                    out_tt(tt)
            for p in range(NPAIR):
                attn_pair(p, 3)
                out_tt(8 + 2 * p)
                out_tt(8 + 2 * p + 1)
            for tt in range(12, NT):
                out_tt(tt)


def _build(causal: bool, repeat: int = 1):
    nc = bacc.Bacc("TRN2", target_bir_lowering=False, debug=False)
    nc.dram_tensors = {}
    nc.dram_tensors["xT"] = nc.dram_tensor("xT", [D, T], BF16, kind="ExternalInput")
    nc.dram_tensors["w_qkv"] = nc.dram_tensor(
        "w_qkv", [D, 3 * DL], BF16, kind="ExternalInput"
    )
    nc.dram_tensors["b_qkv"] = nc.dram_tensor(
        "b_qkv", [3 * DL], F32, kind="ExternalInput"
    )
    nc.dram_tensors["w_out"] = nc.dram_tensor(
        "w_out", [DL, D], BF16, kind="ExternalInput"
    )
    nc.dram_tensors["out"] = nc.dram_tensor("out", [T, D], F32, kind="ExternalOutput")
    with tile.TileContext(nc) as tc:
        for _rep in range(repeat):
            _emit(nc, tc, causal)
    nc.compile()
    return nc


def _get_program(causal: bool):
    key = ("prog", causal)
    if key not in _CACHED:
        _CACHED[key] = _build(causal)
    return _CACHED[key]


def _run_fast(nc, causal, in_maps):
    """Execute via a cached jitted shard_map executable."""
    try:
        import jax
        from jax.sharding import Mesh, NamedSharding, PartitionSpec
        from jax.experimental.shard_map import shard_map
        from concourse import bass2jax
        from concourse.bass2jax import _bass_exec_p, install_neuronx_cc_hook

        key = ("exec", causal)
        if key not in _CACHED:
            install_neuronx_cc_hook()
            partition_name = (
                nc.partition_id_tensor.name if nc.partition_id_tensor else None
            )
            in_names, out_names, out_avals, zero_outs = [], [], [], []
            for alloc in nc.m.functions[0].allocations:
                if not isinstance(alloc, mybir.MemoryLocationSet):
                    continue
                name = alloc.memorylocations[0].name
                if alloc.kind == "ExternalInput":
                    if name != partition_name:
                        in_names.append(name)
                elif alloc.kind == "ExternalOutput":
                    out_names.append(name)
                    shape = tuple(alloc.tensor_shape)
                    dtype = mybir.dt.np(alloc.dtype)
                    out_avals.append(jax.core.ShapedArray(shape, dtype))
                    zero_outs.append(np.zeros(shape, dtype))
            n_params = len(in_names)
            in_names_full = in_names + out_names + (
                [partition_name] if partition_name else []
            )

            def _body(*args):
                operands = list(args)
                if partition_name is not None:
                    operands.append(bass2jax.partition_id_tensor())
                return tuple(
                    _bass_exec_p.bind(
                        *operands,
                        out_avals=tuple(out_avals),
                        in_names=tuple(in_names_full),
                        out_names=tuple(out_names),
                        lowering_input_output_aliases=(),
                        sim_require_finite=True,
                        sim_require_nnan=True,
                        nc=nc,
                    )
                )

            devices = jax.devices()[:8]
            mesh = Mesh(np.asarray(devices), ("core",))
            ex = jax.jit(
                shard_map(
                    _body,
                    mesh=mesh,
                    in_specs=(PartitionSpec("core"),) * (n_params + len(out_names)),
                    out_specs=(PartitionSpec("core"),) * len(out_names),
                    check_rep=False,
                ),
                keep_unused=True,
            )
            _CACHED[key] = (ex, in_names, zero_outs, mesh)
        ex, in_names, zero_outs, mesh = _CACHED[key]
        sh = NamedSharding(mesh, PartitionSpec("core"))
        concat_in = [
            np.concatenate([np.asarray(m[nm]) for m in in_maps], axis=0)
            for nm in in_names
        ]
        concat_zeros = [
            np.zeros((8 * z.shape[0], *z.shape[1:]), z.dtype) for z in zero_outs
        ]
        dev = [jax.device_put(a, sh) for a in concat_in + concat_zeros]
        out_arrs = ex(*dev)
        full = np.asarray(out_arrs[0]).reshape(8, T, D)
        return [full[c] for c in range(8)]
    except Exception:
        res = run_bass_kernel_spmd(nc, in_maps, list(range(8)))
        return [r["out"] for r in res.results]


def kernel(x, attn_mask, W_qkv, b_qkv, W_out, b_out, causal):
    x = np.asarray(x, dtype=np.float32)
    W_qkv = np.asarray(W_qkv, dtype=np.float32)
    b_qkv_np = np.asarray(b_qkv, dtype=np.float32)
    W_out = np.asarray(W_out, dtype=np.float32)
    b_out = np.asarray(b_out, dtype=np.float32)
    causal = bool(int(causal))

    nc = _get_program(causal)

    bf16 = ml_dtypes.bfloat16
    shards = []
    for g in range(2):
        w_shard = np.ascontiguousarray(
            np.concatenate(
                [
                    W_qkv[:, g * DL : (g + 1) * DL],
                    W_qkv[:, D + g * DL : D + (g + 1) * DL],
                    W_qkv[:, 2 * D + g * DL : 2 * D + (g + 1) * DL],
                ],
                axis=1,
            ).astype(bf16)
        )
        b_shard = np.ascontiguousarray(
            np.concatenate(
                [
                    b_qkv_np[g * DL : (g + 1) * DL],
                    b_qkv_np[D + g * DL : D + (g + 1) * DL],
                    b_qkv_np[2 * D + g * DL : 2 * D + (g + 1) * DL],
                ]
            )
        )
        wo_shard = np.ascontiguousarray(
            W_out[g * DL : (g + 1) * DL, :].astype(bf16)
        )
        shards.append((w_shard, b_shard, wo_shard))

    xT_b = [np.ascontiguousarray(x[b].T.astype(bf16)) for b in range(B)]
    in_maps = []
    for c in range(8):
        b = c % B
        g = c // B
        w_shard, b_shard, wo_shard = shards[g]
        in_maps.append(
            {
                "xT": xT_b[b],
                "w_qkv": w_shard,
                "b_qkv": b_shard,
                "w_out": wo_shard,
            }
        )

    outs = _run_fast(nc, causal, in_maps)
    y = np.empty((B, T, D), dtype=np.float32)
    for b in range(B):
        y[b] = outs[b] + outs[B + b] + b_out
    return y
